# revision 18
# baseline (speedup 1.0000x reference)
"""CConv (continuous conv / GNN message passing) Trainium2 Bass kernel.

Math (per point n):
    pf[n,m,:]  = feat_in[neighbor_idx[n,m], :]                 # gather
    t[n,s,i]   = sum_m select_mat[n,m,s] * pf[n,m,i]           # stage 1
    out[n,o]   = sum_{s,i} t[n,s,i] * W[s,o,i]                 # stage 2

Strategy: data-parallel over points across 8 cores; per core, groups of 128
points (32 blocks of 4 points). The neighbor gather is done host-side (the
walrus indirect-DMA lowering on this toolchain only supports 128 rows per
call, far too slow at 200k rows/core) and shipped as a contiguous bf16
stream. Stage 1 runs as one matmul per 4-point block against a
block-diagonal select operand built on-chip by four DVE tensor_scalar
multiplies (4x perf mode: bf16, even dims, 4B-aligned runs — S padded
27->28); stage 2 contracts (s,i) against the replicated weight with PSUM
accumulation.
"""
import sys

sys.path.insert(0, '/opt/trn_rl_repo')

import numpy as np
import ml_dtypes

import concourse.bass as bass
import concourse.tile as tile
from concourse import bacc, mybir
from concourse.bass_utils import run_bass_kernel_spmd

BF16 = ml_dtypes.bfloat16

N = 50000
M = 32            # neighbors per point
S = 27            # spatial bins
SP = 28           # padded spatial (even, 4B-aligned bf16 runs)
I = 128           # in channels
O = 128           # out channels
NCORES = 8
NPAD = 50176      # 8 * 49 * 128
NPC = NPAD // NCORES        # 6272 points per core
G = NPC // 128              # 49 groups of 128 points
B = 32                      # 4-point blocks per group
SUB = 4                     # blocks accumulated per PSUM bank tile
BD = 4 * SP                 # block-diag columns per block (112)


def build_nc():
    nc = bacc.Bacc("TRN2", target_bir_lowering=False, debug=False)

    pfp = nc.dram_tensor("pfp", [G, 128, B * I], mybir.dt.bfloat16, kind="ExternalInput")
    selp = nc.dram_tensor("selp", [G, 128, B * SP], mybir.dt.bfloat16, kind="ExternalInput")
    wt = nc.dram_tensor("wt", [I, S * O], mybir.dt.bfloat16, kind="ExternalInput")
    maskc = nc.dram_tensor("maskc", [128, BD], mybir.dt.bfloat16, kind="ExternalInput")
    outp = nc.dram_tensor("outp", [NPC, O], mybir.dt.bfloat16, kind="ExternalOutput")

    with tile.TileContext(nc) as tc:
        with (
            tc.tile_pool(name="const", bufs=1) as const_pool,
            tc.tile_pool(name="work", bufs=4) as work,
            tc.tile_pool(name="psum1", bufs=4, space="PSUM") as psum1,
            tc.tile_pool(name="psum2", bufs=2, space="PSUM") as psum2,
        ):
            wt_t = const_pool.tile([128, S * O], mybir.dt.bfloat16)
            nc.sync.dma_start(out=wt_t[:], in_=wt[:])
            mask_t = const_pool.tile([128, BD], mybir.dt.bfloat16)
            nc.sync.dma_start(out=mask_t[:], in_=maskc[:])

            def mask_blocks(eng, sel_t, rhs_t, blk0, nblk):
                # rhs_t[q, blk, s*4+nb] = sel_t[q, blk*SP+s] * mask[q, s*4+nb]
                # over a contiguous (possibly pair-spanning) block range.
                out_ap = bass.AP(tensor=rhs_t.tensor,
                                 offset=rhs_t[:].offset + blk0 * BD,
                                 ap=[rhs_t[:].ap[0], [BD, nblk], [4, SP], [1, 4]])
                in0_ap = bass.AP(tensor=sel_t.tensor,
                                 offset=sel_t[:].offset + blk0 * SP,
                                 ap=[sel_t[:].ap[0], [SP, nblk], [1, SP], [0, 4]])
                in1_ap = bass.AP(tensor=mask_t.tensor, offset=mask_t[:].offset,
                                 ap=[mask_t[:].ap[0], [0, nblk], [4, SP], [1, 4]])
                eng.tensor_tensor(out=out_ap, in0=in0_ap, in1=in1_ap,
                                  op=mybir.AluOpType.mult)

            GPL = 11  # gpsimd-masked blocks per group (fused across pairs)
            for gp in range(0, G, 2):
                npair = min(2, G - gp)
                # paired sel / rhs tiles so one GpSimd op can span the pair
                sel_t = work.tile([128, npair, B * SP], mybir.dt.bfloat16)
                s_in = selp[gp:gp + npair].rearrange("g p c -> p g c")
                nc.sync.dma_start(out=sel_t[:], in_=s_in)
                rhs_t = work.tile([128, npair, B, BD], mybir.dt.bfloat16)
                if npair == 2:
                    # DVE: g0 blocks [0, B-GPL), g1 blocks [GPL, B)
                    mask_blocks(nc.vector, sel_t, rhs_t, 0, B - GPL)
                    mask_blocks(nc.gpsimd, sel_t, rhs_t, B - GPL, 2 * GPL)
                    mask_blocks(nc.vector, sel_t, rhs_t, B + GPL, B - GPL)
                else:
                    mask_blocks(nc.vector, sel_t, rhs_t, 0, B - GPL)
                    mask_blocks(nc.gpsimd, sel_t, rhs_t, B - GPL, GPL)

                for h in range(npair):
                    g = gp + h
                    pf_t = work.tile([128, B, I], mybir.dt.bfloat16)
                    # alternate HWDGE rings (SP / ACT) for the big loads
                    if g % 2 == 0:
                        nc.sync.dma_start(out=pf_t[:], in_=pfp[g])
                    else:
                        nc.scalar.dma_start(out=pf_t[:], in_=pfp[g])

                    SW = 108  # psum cols per block (pad dropped)
                    Tg = work.tile([128, S * 128], mybir.dt.bfloat16)
                    for c in range(B // SUB):
                        pt = psum1.tile([128, SUB * SW], mybir.dt.float32, space="PSUM")
                        for sub in range(SUB):
                            b = c * SUB + sub
                            nc.tensor.matmul(
                                out=pt[:, sub * SW:(sub + 1) * SW],
                                lhsT=pf_t[:, b, :],
                                rhs=rhs_t[:, h, b, 0:SW],
                                start=True, stop=True,
                            )
                        # scatter-copy to Tg: dst col = s*128 + (c*SUB+sub)*4 + nb
                        src_ap = bass.AP(tensor=pt.tensor, offset=pt[:].offset,
                                         ap=[pt[:].ap[0], [SW, SUB], [4, S], [1, 4]])
                        dst_ap = bass.AP(tensor=Tg.tensor, offset=Tg[:].offset + c * SUB * 4,
                                         ap=[Tg[:].ap[0], [4, SUB], [128, S], [1, 4]])
                        if c % 8 < 2:
                            nc.vector.tensor_copy(out=dst_ap, in_=src_ap)
                        else:
                            nc.scalar.copy(out=dst_ap, in_=src_ap)

                    po = psum2.tile([128, O], mybir.dt.float32, space="PSUM")
                    for s in range(S):
                        nc.tensor.matmul(
                            out=po[:],
                            lhsT=Tg[:, s * 128:(s + 1) * 128],
                            rhs=wt_t[:, s * O:(s + 1) * O],
                            start=(s == 0), stop=(s == S - 1),
                        )
                    ot = work.tile([128, O], mybir.dt.bfloat16)
                    nc.scalar.copy(out=ot[:], in_=po[:])
                    nc.sync.dma_start(out=outp[g * 128:(g + 1) * 128, :], in_=ot[:])

    nc.compile()
    return nc


_NC = None


def get_nc():
    global _NC
    if _NC is None:
        _NC = build_nc()
    return _NC


def make_in_maps(feat_in, select_mat, weight, neighbor_idx):
    featb_np = np.asarray(feat_in, dtype=np.float32).astype(BF16)

    sel = np.asarray(select_mat, dtype=np.float32)
    sel_pad = np.zeros((NPAD, M, SP), dtype=np.float32)
    sel_pad[:N, :, :S] = sel

    nidx = np.asarray(neighbor_idx).astype(np.int64)
    idx_pad = np.zeros((NPAD, M), dtype=np.int64)
    idx_pad[:N] = nidx

    w = np.asarray(weight, dtype=np.float32)
    wt_np = np.ascontiguousarray(
        w.reshape(S, O, I).transpose(2, 0, 1).reshape(I, S * O)).astype(BF16)

    q = np.arange(128)[:, None]
    c = np.arange(BD)[None, :]
    mask_np = (q // 32 == c % 4).astype(BF16)

    in_maps = []
    for core in range(NCORES):
        lo = core * NPC
        selc = sel_pad[lo:lo + NPC]
        idxc = idx_pad[lo:lo + NPC]
        # selp[g, nb*32+m, b*SP+s] = sel[g*128 + b*4 + nb, m, s]
        selp_np = np.ascontiguousarray(
            selc.reshape(G, B, 4, M, SP).transpose(0, 2, 3, 1, 4)
        ).reshape(G, 128, B * SP).astype(BF16)
        # idxp[g, nb*32+m, b] = neighbor_idx[g*128 + b*4 + nb, m]
        idxp = np.ascontiguousarray(
            idxc.reshape(G, B, 4, M).transpose(0, 2, 3, 1))  # [G, 128, B]
        # host gather: pfp[g, q, b, :] = featb[idxp[g, q, b]]
        pfp_np = featb_np[idxp].reshape(G, 128, B * I)
        in_maps.append({
            "pfp": pfp_np,
            "selp": selp_np,
            "wt": wt_np,
            "maskc": mask_np,
        })
    return in_maps


def run(feat_in, select_mat, weight, neighbor_idx, trace=False):
    nc = get_nc()
    in_maps = make_in_maps(feat_in, select_mat, weight, neighbor_idx)
    res = run_bass_kernel_spmd(nc, in_maps, core_ids=list(range(NCORES)), trace=trace)
    outs = [res.results[c]["outp"] for c in range(NCORES)]
    full = np.concatenate(outs, axis=0)[:N].astype(np.float32)   # [N, O]
    return full[:, :, None], res


def kernel(feat_in, select_mat, weight, neighbor_idx):
    out, _ = run(feat_in, select_mat, weight, neighbor_idx, trace=False)
    return out


# revision 19
# speedup vs baseline: 1.4115x; 1.4115x over previous
"""CConv (continuous conv / GNN message passing) Trainium2 Bass kernel.

Math (per point n):
    pf[n,m,:]  = feat_in[neighbor_idx[n,m], :]                 # gather
    t[n,s,i]   = sum_m select_mat[n,m,s] * pf[n,m,i]           # stage 1
    out[n,o]   = sum_{s,i} t[n,s,i] * W[s,o,i]                 # stage 2

Strategy: data-parallel over points across 8 cores; per core, groups of 128
points (32 blocks of 4 points). The neighbor gather is done host-side (the
walrus indirect-DMA lowering on this toolchain only supports 128 rows per
call, far too slow at 200k rows/core) and shipped as a contiguous bf16
stream. Stage 1 runs as one matmul per 4-point block against a
block-diagonal select operand built on-chip by four DVE tensor_scalar
multiplies (4x perf mode: bf16, even dims, 4B-aligned runs — S padded
27->28); stage 2 contracts (s,i) against the replicated weight with PSUM
accumulation.
"""
import sys

sys.path.insert(0, '/opt/trn_rl_repo')

import numpy as np
import ml_dtypes

import concourse.bass as bass
import concourse.tile as tile
from concourse import bacc, mybir
from concourse.bass_utils import run_bass_kernel_spmd

BF16 = ml_dtypes.bfloat16

N = 50000
M = 32            # neighbors per point
S = 27            # spatial bins
SP = 28           # padded spatial (even, 4B-aligned bf16 runs)
I = 128           # in channels
O = 128           # out channels
NCORES = 8
NPAD = 50176      # 8 * 49 * 128
NPC = NPAD // NCORES        # 6272 points per core
G = NPC // 128              # 49 groups of 128 points
B = 32                      # 4-point blocks per group
SUB = 4                     # blocks accumulated per PSUM bank tile
BD = 4 * SP                 # block-diag columns per block (112)


def build_nc():
    nc = bacc.Bacc("TRN2", target_bir_lowering=False, debug=False)

    pfp = nc.dram_tensor("pfp", [G, 128, B * I], mybir.dt.bfloat16, kind="ExternalInput")
    selp = nc.dram_tensor("selp", [G, 128, B * SP], mybir.dt.bfloat16, kind="ExternalInput")
    wt = nc.dram_tensor("wt", [I, S * O], mybir.dt.bfloat16, kind="ExternalInput")
    maskc = nc.dram_tensor("maskc", [128, BD], mybir.dt.bfloat16, kind="ExternalInput")
    outp = nc.dram_tensor("outp", [NPC, O], mybir.dt.bfloat16, kind="ExternalOutput")

    with tile.TileContext(nc) as tc:
        with (
            tc.tile_pool(name="const", bufs=1) as const_pool,
            tc.tile_pool(name="work", bufs=4) as work,
            tc.tile_pool(name="psum1", bufs=4, space="PSUM") as psum1,
            tc.tile_pool(name="psum2", bufs=2, space="PSUM") as psum2,
        ):
            wt_t = const_pool.tile([128, S * O], mybir.dt.bfloat16)
            nc.sync.dma_start(out=wt_t[:], in_=wt[:])
            mask_t = const_pool.tile([128, BD], mybir.dt.bfloat16)
            nc.sync.dma_start(out=mask_t[:], in_=maskc[:])

            def mask_blocks(eng, sel_t, rhs_t, blk0, nblk):
                # rhs_t[q, blk, s*4+nb] = sel_t[q, blk*SP+s] * mask[q, s*4+nb]
                # over a contiguous (possibly pair-spanning) block range.
                out_ap = bass.AP(tensor=rhs_t.tensor,
                                 offset=rhs_t[:].offset + blk0 * BD,
                                 ap=[rhs_t[:].ap[0], [BD, nblk], [4, SP], [1, 4]])
                in0_ap = bass.AP(tensor=sel_t.tensor,
                                 offset=sel_t[:].offset + blk0 * SP,
                                 ap=[sel_t[:].ap[0], [SP, nblk], [1, SP], [0, 4]])
                in1_ap = bass.AP(tensor=mask_t.tensor, offset=mask_t[:].offset,
                                 ap=[mask_t[:].ap[0], [0, nblk], [4, SP], [1, 4]])
                eng.tensor_tensor(out=out_ap, in0=in0_ap, in1=in1_ap,
                                  op=mybir.AluOpType.mult)

            GPL = 11  # gpsimd-masked blocks per group
            for gp in range(G):
                npair = 1
                sel_t = work.tile([128, npair, B * SP], mybir.dt.bfloat16)
                s_in = selp[gp:gp + npair].rearrange("g p c -> p g c")
                nc.sync.dma_start(out=sel_t[:], in_=s_in)
                rhs_t = work.tile([128, npair, B, BD], mybir.dt.bfloat16)
                mask_blocks(nc.vector, sel_t, rhs_t, 0, B - GPL)
                mask_blocks(nc.gpsimd, sel_t, rhs_t, B - GPL, GPL)

                for h in range(npair):
                    g = gp + h
                    pf_t = work.tile([128, B, I], mybir.dt.bfloat16)
                    # alternate HWDGE rings (SP / ACT) for the big loads
                    if g % 2 == 0:
                        nc.sync.dma_start(out=pf_t[:], in_=pfp[g])
                    else:
                        nc.scalar.dma_start(out=pf_t[:], in_=pfp[g])

                    SW = 108  # psum cols per block (pad dropped)
                    Tg = work.tile([128, S * 128], mybir.dt.bfloat16)
                    for c in range(B // SUB):
                        pt = psum1.tile([128, SUB * SW], mybir.dt.float32, space="PSUM")
                        for sub in range(SUB):
                            b = c * SUB + sub
                            nc.tensor.matmul(
                                out=pt[:, sub * SW:(sub + 1) * SW],
                                lhsT=pf_t[:, b, :],
                                rhs=rhs_t[:, h, b, 0:SW],
                                start=True, stop=True,
                            )
                        # scatter-copy to Tg: dst col = s*128 + (c*SUB+sub)*4 + nb
                        src_ap = bass.AP(tensor=pt.tensor, offset=pt[:].offset,
                                         ap=[pt[:].ap[0], [SW, SUB], [4, S], [1, 4]])
                        dst_ap = bass.AP(tensor=Tg.tensor, offset=Tg[:].offset + c * SUB * 4,
                                         ap=[Tg[:].ap[0], [4, SUB], [128, S], [1, 4]])
                        if c % 8 < 2:
                            nc.vector.tensor_copy(out=dst_ap, in_=src_ap)
                        else:
                            nc.scalar.copy(out=dst_ap, in_=src_ap)

                    po = psum2.tile([128, O], mybir.dt.float32, space="PSUM")
                    for s in range(S):
                        nc.tensor.matmul(
                            out=po[:],
                            lhsT=Tg[:, s * 128:(s + 1) * 128],
                            rhs=wt_t[:, s * O:(s + 1) * O],
                            start=(s == 0), stop=(s == S - 1),
                        )
                    ot = work.tile([128, O], mybir.dt.bfloat16)
                    nc.scalar.copy(out=ot[:], in_=po[:])
                    nc.sync.dma_start(out=outp[g * 128:(g + 1) * 128, :], in_=ot[:])

    nc.compile()
    return nc


_NC = None


def get_nc():
    global _NC
    if _NC is None:
        _NC = build_nc()
    return _NC


def make_in_maps(feat_in, select_mat, weight, neighbor_idx):
    featb_np = np.asarray(feat_in, dtype=np.float32).astype(BF16)

    sel = np.asarray(select_mat, dtype=np.float32)
    sel_pad = np.zeros((NPAD, M, SP), dtype=np.float32)
    sel_pad[:N, :, :S] = sel

    nidx = np.asarray(neighbor_idx).astype(np.int64)
    idx_pad = np.zeros((NPAD, M), dtype=np.int64)
    idx_pad[:N] = nidx

    w = np.asarray(weight, dtype=np.float32)
    wt_np = np.ascontiguousarray(
        w.reshape(S, O, I).transpose(2, 0, 1).reshape(I, S * O)).astype(BF16)

    q = np.arange(128)[:, None]
    c = np.arange(BD)[None, :]
    mask_np = (q // 32 == c % 4).astype(BF16)

    in_maps = []
    for core in range(NCORES):
        lo = core * NPC
        selc = sel_pad[lo:lo + NPC]
        idxc = idx_pad[lo:lo + NPC]
        # selp[g, nb*32+m, b*SP+s] = sel[g*128 + b*4 + nb, m, s]
        selp_np = np.ascontiguousarray(
            selc.reshape(G, B, 4, M, SP).transpose(0, 2, 3, 1, 4)
        ).reshape(G, 128, B * SP).astype(BF16)
        # idxp[g, nb*32+m, b] = neighbor_idx[g*128 + b*4 + nb, m]
        idxp = np.ascontiguousarray(
            idxc.reshape(G, B, 4, M).transpose(0, 2, 3, 1))  # [G, 128, B]
        # host gather: pfp[g, q, b, :] = featb[idxp[g, q, b]]
        pfp_np = featb_np[idxp].reshape(G, 128, B * I)
        in_maps.append({
            "pfp": pfp_np,
            "selp": selp_np,
            "wt": wt_np,
            "maskc": mask_np,
        })
    return in_maps


def run(feat_in, select_mat, weight, neighbor_idx, trace=False):
    nc = get_nc()
    in_maps = make_in_maps(feat_in, select_mat, weight, neighbor_idx)
    res = run_bass_kernel_spmd(nc, in_maps, core_ids=list(range(NCORES)), trace=trace)
    outs = [res.results[c]["outp"] for c in range(NCORES)]
    full = np.concatenate(outs, axis=0)[:N].astype(np.float32)   # [N, O]
    return full[:, :, None], res


def kernel(feat_in, select_mat, weight, neighbor_idx):
    out, _ = run(feat_in, select_mat, weight, neighbor_idx, trace=False)
    return out
